# revision 68
# baseline (speedup 1.0000x reference)
"""ConvolvedAttention (sliding-window causal attention, W=33) on 8 TRN2 NeuronCores.

Sequence L=8192 split 8 ways (1024 tokens/core), batch N=2 handled per core.
All matmuls in bf16. Attention is tiled as stride-96 query tiles against
128-key windows so each query's full 33-key causal band lives in a single
tile: no cross-tile softmax combining.

Scores use a block-diagonal zero-padded q layout (q_z): one
[K=128, M=128 keys, N=4*96] FWL-eligible matmul per (unit, E'-half)
computes all 4 heads of that half at once (cross-head contraction terms
multiply zeros), replacing 16 small row-group-tiled matmuls per unit that
each paid ~200ns of isolated fill/drain latency. q_z is built by zeroing
once (GpSimd/DVE memsets during the input-DMA window) and DMA-scattering
four 32-row diagonal strips per E'-half, split across the Act and sync HW
DGE queues; the q-projection PSUM is evacuated in bank-parallel halves on
Act and DVE so the strips aren't gated on a serial evacuation chain.

Phase B is software-pipelined at (tile, batch) "unit" granularity: the PE
stream is skewed so unit u's scores run two units ahead of its AV matmuls,
keeping the tensor engine dense (warm HAM clock, fill/drain overlapped)
while Act does exp and DVE does masking/normalization for the in-flight
units. PSUM: scores 2 bufs x 2 banks, AV(+band sums) 2 bufs x 1 bank,
V-proj 1 bank, out-proj 1 bank = 8 banks exactly. Output returns as bf16
and is upcast + bias-folded on host.
"""

import numpy as np

# ---- problem constants (hardcoded per contract) ----
L, N, E = 8192, 2, 256
H, HD = 8, 32
WHALF = 32            # window//2 ; attended span = 33 (past only)
NCORES = 8
T = L // NCORES       # 1024 tokens per core per batch entry
PAD = 32              # left halo / right zero-pad on k/v
TLP = PAD + T + PAD   # 1088 padded local k/v tokens
QT = 96               # query-tile width
NT = 11               # tiles per batch entry (10x96 + 64)
NU = 2 * NT           # pipeline units: (tile, batch)

# wpack_bf column layout (bf16 cols per partition)
_WQ = 0               # 4 tiles [128,128]  (ki*2+ko)
_WK = 512             # 4 tiles [128,128]
_WV = 1024            # 2 tiles [128,256]  (ki)
_WO = 1536            # 2 tiles [128,256]  (g = E_in chunk)
_ONES = 2048          # [128,32] all-ones (sums lhsT)
_WBF_COLS = 2080
# separate mask param: [128, 2*768]
_BAND = 0             # [128,8*96] band mask replicated per slot (t>0)
_BAND0 = 768          # [128,8*96] first-tile mask (per-core content)
_WBM_COLS = 1536

_STATE = {}


def _build_program():
    import concourse.bacc as bacc
    import concourse.tile as tile
    import concourse.mybir as mybir
    from contextlib import ExitStack

    f32 = mybir.dt.float32
    bf16 = mybir.dt.bfloat16
    AF = mybir.ActivationFunctionType

    nc = bacc.Bacc("TRN2", target_bir_lowering=False, debug=False)
    xq_d = nc.declare_dram_parameter("xq", [2, 128, N * T], bf16, isOutput=False)
    xk_d = nc.declare_dram_parameter("xk", [2, 128, N * TLP], bf16, isOutput=False)
    xv_d = nc.declare_dram_parameter("xv", [2, 128, N * TLP], bf16, isOutput=False)
    wb_d = nc.declare_dram_parameter("wb", [128, _WBF_COLS], bf16, isOutput=False)
    wm_d = nc.declare_dram_parameter("wm", [128, _WBM_COLS], bf16, isOutput=False)
    wf_d = nc.declare_dram_parameter("wf", [128, 2], f32, isOutput=False)
    out_d = nc.declare_dram_parameter("out", [NT, QT, N, E], bf16, isOutput=True)

    with ExitStack() as stk:
        tc = stk.enter_context(tile.TileContext(nc))
        sb = stk.enter_context(tc.tile_pool(name="sb", bufs=1))
        sb_pr = stk.enter_context(tc.tile_pool(name="pr", bufs=3))
        sb_pm = stk.enter_context(tc.tile_pool(name="pm", bufs=3))
        sb_v = stk.enter_context(tc.tile_pool(name="vt", bufs=3))
        sb_sr = stk.enter_context(tc.tile_pool(name="sr", bufs=2))
        sb_av = stk.enter_context(tc.tile_pool(name="avn", bufs=5))
        sb_o = stk.enter_context(tc.tile_pool(name="osb", bufs=2))

        # ---- input loads (ordered by first use: weights, q, k, masks, v) ----
        # wb in two transfers so q-proj matmuls only wait for the WQ block
        wb = sb.tile([128, _WBF_COLS], bf16, tag="wb")
        nc.sync.dma_start(wb[:, :_WK], wb_d[:, :_WK])
        nc.sync.dma_start(wb[:, _WK:], wb_d[:, _WK:])
        wf = sb.tile([128, 2], f32, tag="wf")
        nc.sync.dma_start(wf[:], wf_d[:])
        xq, xk, xv = [], [], []
        for ki in range(2):
            t_q = sb.tile([128, N * T], bf16, tag=f"xq{ki}", name=f"xq{ki}")
            nc.sync.dma_start(t_q[:, :1024], xq_d[ki, :, :1024])
            nc.sync.dma_start(t_q[:, 1024:], xq_d[ki, :, 1024:])
            xq.append(t_q)
        for ki in range(2):
            t_k = sb.tile([128, N * TLP], bf16, tag=f"xk{ki}", name=f"xk{ki}")
            nc.sync.dma_start(t_k[:], xk_d[ki])
            xk.append(t_k)
        wm = sb.tile([128, _WBM_COLS], bf16, tag="wm")
        nc.sync.dma_start(wm[:], wm_d[:])
        for ki in range(2):
            t_v = sb.tile([128, N * TLP], bf16, tag=f"xv{ki}", name=f"xv{ki}")
            nc.sync.dma_start(t_v[:], xv_d[ki])
            xv.append(t_v)

        q_sb = [sb.tile([128, N * T], bf16, tag=f"q{ko}", name=f"q{ko}") for ko in range(2)]
        k_sb = [sb.tile([128, N * TLP], bf16, tag=f"k{ko}", name=f"k{ko}") for ko in range(2)]

        # block-diagonal zero-padded q: q_z[ch][p, s*2048 + n*1024 + tau]
        # holds q_sb[ch][p, n*1024 + tau] iff p//32 == s, else 0. Lets one
        # [K=128, M=128, N=4*qw] matmul compute scores for 4 heads at once.
        # s-major layout keeps each diagonal strip contiguous per partition
        # (one 4KB DMA run). Zeroed during the input-DMA window; the q strips
        # then overwrite the diagonal.
        # (warm scratch memset first: it must not queue behind the big q_z
        # memset on DVE, or the PE warm-up below would stall on it)
        warm = sb.tile([128, 64], bf16, tag="warm")
        nc.vector.memset(warm[:], 0.0)

        q_z = [sb.tile([128, 2 * 4 * T], bf16, tag=f"qz{ch}", name=f"qz{ch}") for ch in range(2)]
        nc.gpsimd.memset(q_z[0][:], 0.0)
        nc.vector.memset(q_z[1][:], 0.0)

        # PE warm-up during the input-DMA window: a burst of dead matmuls on
        # a scratch tile keeps the HAM activity window busy so phase A's
        # matmuls start at the warm 2.4GHz clock instead of 1.2GHz (the HAM
        # un-throttles after ~3.4us of sustained PE activity).
        with tc.tile_pool(name="pw", bufs=1, space="PSUM") as pw:
            wps = pw.tile([64, 64], f32, tag="wps", name="wps")
            for _ in range(40):
                nc.tensor.matmul(
                    wps[:], warm[:, :64], warm[:, :64],
                    start=True, stop=True, skip_group_check=True,
                )

        # ---- phase A: q then k projections (1024-col chunks, 2 psum banks) ----
        # q first (both ko) so the q_z strips can start while k projects.
        with tc.tile_pool(name="pp", bufs=4, space="PSUM") as pp:
            for ko in range(2):
                bq_ap = wf[:, ko : ko + 1]
                for g0 in range(0, N * T, 1024):
                    ps = pp.tile([128, 1024], f32, tag="pq", name="pq")
                    for half in range(2):
                        h0 = g0 + half * 512
                        for ki in range(2):
                            nc.tensor.matmul(
                                ps[:, half * 512 : half * 512 + 512],
                                wb[:, _WQ + (ki * 2 + ko) * 128 : _WQ + (ki * 2 + ko + 1) * 128],
                                xq[ki][:, h0 : h0 + 512],
                                start=(ki == 0), stop=(ki == 1),
                                skip_group_check=True,
                            )
                    # evacuate in halves on Act and DVE concurrently (the two
                    # halves are different PSUM banks): the serialized Act
                    # evacuations otherwise gate the q_z strips
                    nc.scalar.activation(
                        q_sb[ko][:, g0 : g0 + 512], ps[:, :512],
                        AF.Identity, bias=bq_ap,
                    )
                    nc.vector.tensor_scalar_add(
                        q_sb[ko][:, g0 + 512 : g0 + 1024], ps[:, 512:], bq_ap
                    )
                # scatter the 32-row diagonal strips of q into q_z, spread
                # over three DMA queue families (Act HW DGE, sync HW DGE,
                # GpSimd SW DGE). Each strip is split into a small head
                # (tokens 0:384, all the first four query tiles read) and the
                # tail, so phase B's first scores aren't gated on the full
                # 1MB strip transfer.
                HT = 384
                for s in range(4):
                    eng = (nc.scalar, nc.sync, nc.gpsimd)[(4 * ko + s) % 3]
                    dst = q_z[ko][32 * s : 32 * s + 32,
                                  2048 * s : 2048 * s + 2048].rearrange(
                        "p (n t) -> p n t", n=2)
                    src = q_sb[ko][32 * s : 32 * s + 32, :].rearrange(
                        "p (n t) -> p n t", n=2)
                    eng.dma_start(dst[:, :, :HT], src[:, :, :HT])
                    eng.dma_start(dst[:, :, HT:], src[:, :, HT:])
            # k projection, ko-interleaved per chunk so early-key evacs land first
            for g0 in range(0, N * TLP, 1024):
                w = min(1024, N * TLP - g0)
                for ko in range(2):
                    ps = pp.tile([128, 1024], f32, tag="pq", name="pq")
                    for half in range(0, w, 512):
                        hw = min(512, w - half)
                        for ki in range(2):
                            nc.tensor.matmul(
                                ps[:, half : half + hw],
                                wb[:, _WK + (ki * 2 + ko) * 128 : _WK + (ki * 2 + ko + 1) * 128],
                                xk[ki][:, g0 + half : g0 + half + hw],
                                start=(ki == 0), stop=(ki == 1),
                                skip_group_check=True,
                            )
                    # evacuate in bank-parallel halves (Act + DVE), matching
                    # the q-evacuation split
                    if w > 512:
                        nc.scalar.copy(k_sb[ko][:, g0 : g0 + 512], ps[:, :512])
                        nc.vector.tensor_copy(
                            k_sb[ko][:, g0 + 512 : g0 + w], ps[:, 512:w]
                        )
                    else:
                        nc.vector.tensor_copy(k_sb[ko][:, g0 : g0 + w], ps[:, :w])

        # ---- phase B: software-pipelined attention units ----
        ones32 = wb[:, _ONES : _ONES + 32]
        v_tiles = {}        # t -> v_t sbuf tile
        pm_tiles = {}       # u -> masked probs sbuf tile
        avn_tiles = {}      # u -> normalized av sbuf tile
        qw_of = lambda t: min(QT, T - QT * t)

        with (
            tc.tile_pool(name="psc", bufs=2, space="PSUM") as psc,
            tc.tile_pool(name="pav", bufs=2, space="PSUM") as pav,
            tc.tile_pool(name="pvo", bufs=1, space="PSUM") as pvo,
            tc.tile_pool(name="pout", bufs=1, space="PSUM") as pout,
        ):
            def emit_vo(t):
                # V projection for both batch entries: [128 keys, 2*256]
                q0 = QT * t
                vo = pvo.tile([128, 512], f32, tag="vo", name="vo")
                for n in range(2):
                    for ki in range(2):
                        nc.tensor.matmul(
                            vo[:, 256 * n : 256 * n + 256],
                            xv[ki][:, n * TLP + q0 : n * TLP + q0 + 128],
                            wb[:, _WV + ki * 256 : _WV + (ki + 1) * 256],
                            start=(ki == 0), stop=(ki == 1),
                            skip_group_check=True,
                        )
                v_t = sb_v.tile([128, 512], bf16, tag="vt", name="vt")
                if t % 2 == 0:
                    nc.scalar.copy(v_t[:], vo[:])
                else:
                    nc.vector.tensor_copy(v_t[:], vo[:])
                v_tiles[t] = v_t

            def emit_sc(u):
                # scores for unit u = (t, n): 8 head slots of 128 psum cols
                t, n = divmod(u, 2)
                q0 = QT * t
                qw = qw_of(t)
                sc = psc.tile([128, 1024], f32, tag="sc", name="sc")
                # one [K=128, M=128, N=4*qw] matmul per E'-half computes all
                # 4 heads of that half: q_z is block-diagonal so cross-head
                # contraction terms multiply zeros. Head h=4*ch+s lands at
                # psum cols 512*ch + qw*s — slots packed contiguously (PSUM
                # has 8-byte cachelines; contiguous out APs run faster) so
                # exp reads two long runs instead of 8 strided slots.
                for ch in range(2):
                    nc.tensor.matmul(
                        sc[:, 512 * ch : 512 * ch + 4 * qw].rearrange(
                            "p (s c) -> p s c", s=4
                        ),
                        k_sb[ch][:, n * TLP + q0 : n * TLP + q0 + 128],
                        q_z[ch][:].rearrange(
                            "p (s n t) -> p s n t", s=4, n=2
                        )[:, :, n, q0 : q0 + qw],
                        start=True, stop=True,
                        skip_group_check=True,
                    )
                # exp over all 8 slots in one Act instruction; probs slot h
                # sits at cols qw*h
                probs = sb_pr.tile([128, 8 * QT], bf16, tag="probs", name="probs")
                sc3 = sc[:].rearrange("p (b c) -> p b c", b=2)[:, :, : 4 * qw]
                pr3 = probs[:, : 8 * qw].rearrange("p (b c) -> p b c", b=2)
                nc.scalar.activation(pr3, sc3, AF.Exp)
                # band mask, multiplicative (invalid probs -> 0)
                b0 = _BAND0 if t == 0 else _BAND
                pm = sb_pm.tile([128, 8 * QT], bf16, tag="pmm", name="pmm")
                if qw == QT:
                    nc.vector.tensor_mul(
                        pm[:], probs[:], wm[:, b0 : b0 + 8 * QT]
                    )
                else:
                    pm3 = pm[:, : 8 * qw].rearrange("p (s c) -> p s c", s=8)
                    pr3m = probs[:, : 8 * qw].rearrange("p (s c) -> p s c", s=8)
                    b3 = wm[:, b0 : b0 + 8 * QT].rearrange(
                        "p (s c) -> p s c", s=8
                    )[:, :, :qw]
                    nc.vector.tensor_mul(pm3, pr3m, b3)
                pm_tiles[u] = pm

            def emit_av(u):
                # AV + band sums; av g at cols 128g, sums g at 256+128g
                t, n = divmod(u, 2)
                qw = qw_of(t)
                pm = pm_tiles.pop(u)
                v_t = v_tiles[t]
                av = pav.tile([128, 512], f32, tag="av", name="av")
                for h in range(H):
                    hb = h % 4
                    g = h // 4
                    pr_s = pm[:, qw * h : qw * h + qw]
                    nc.tensor.matmul(
                        av[32 * hb : 32 * hb + 32, 128 * g : 128 * g + qw],
                        v_t[:, 256 * n + 32 * h : 256 * n + 32 * h + 32],
                        pr_s,
                        start=True, stop=True,
                        tile_position=(0, 32 * hb), skip_group_check=True,
                    )
                    nc.tensor.matmul(
                        av[32 * hb : 32 * hb + 32, 256 + 128 * g : 256 + 128 * g + qw],
                        ones32,
                        pr_s,
                        start=True, stop=True,
                        tile_position=(0, 32 * hb), skip_group_check=True,
                    )
                # reciprocal of sums, then normalize av -> avn (bf16)
                s_r = sb_sr.tile([128, 2 * QT], f32, tag="sr", name="sr")
                sums3 = av[:, 256:].rearrange("p (s c) -> p s c", s=2)[:, :, :qw]
                sr3 = s_r[:].rearrange("p (s c) -> p s c", s=2)[:, :, :qw]
                nc.vector.reciprocal_approx_fast(out=sr3, in_=sums3)
                avn = sb_av.tile([128, 2 * QT], bf16, tag="avn", name="avn")
                av3 = av[:, :256].rearrange("p (s c) -> p s c", s=2)[:, :, :qw]
                avn3 = avn[:].rearrange("p (s c) -> p s c", s=2)[:, :, :qw]
                nc.vector.tensor_mul(avn3, av3, sr3)
                avn_tiles[u] = avn

            def emit_po(t):
                # out projection: [qw, 2*256], then evacuate + DMA
                qw = qw_of(t)
                po = pout.tile([QT, 512], f32, tag="po", name="po")
                for n in range(2):
                    avn = avn_tiles.pop(2 * t + n)
                    for g in range(2):
                        nc.tensor.matmul(
                            po[:qw, 256 * n : 256 * n + 256],
                            avn[:, QT * g : QT * g + qw],
                            wb[:, _WO + g * 256 : _WO + (g + 1) * 256],
                            start=(g == 0), stop=(g == 1),
                            skip_group_check=True,
                        )
                o_sb = sb_o.tile([QT, 512], bf16, tag="osb", name="osb")
                if t % 2 == 0:
                    nc.vector.tensor_copy(o_sb[:qw], po[:qw])
                else:
                    nc.scalar.copy(o_sb[:qw], po[:qw])
                nc.sync.dma_start(out_d[t, :qw], o_sb[:qw])

            # software pipeline: sc_u runs 2 units ahead of av_u;
            # po_t directly follows av_{2t+2}.
            for u in range(NU + 5):
                if u % 2 == 0 and u // 2 < NT:
                    emit_vo(u // 2)
                if u < NU:
                    emit_sc(u)
                if 0 <= u - 2 < NU:
                    emit_av(u - 2)
                if u % 2 == 0 and 0 <= (u - 4) // 2 < NT:
                    emit_po((u - 4) // 2)
    nc.compile()
    return nc


def _host_prep(query, key, value, in_proj_w, in_proj_b, out_proj_w, out_proj_b):
    import ml_dtypes

    bf = ml_dtypes.bfloat16
    s = 1.0 / np.sqrt(HD)
    wq = (in_proj_w[:E] * s).astype(np.float32)
    bq = (in_proj_b[:E] * s).astype(np.float32)
    wk = in_proj_w[E : 2 * E].astype(np.float32)
    wv = in_proj_w[2 * E :].astype(np.float32)
    bv = in_proj_b[2 * E :].astype(np.float32)
    wo = out_proj_w.astype(np.float32)

    wb_base = np.zeros((128, _WBF_COLS), np.float32)
    wqT, wkT = wq.T.copy(), wk.T.copy()   # [E_in, E_out]
    for ki in range(2):
        for ko in range(2):
            wb_base[:, _WQ + (ki * 2 + ko) * 128 : _WQ + (ki * 2 + ko + 1) * 128] = \
                wqT[ki * 128 : (ki + 1) * 128, ko * 128 : (ko + 1) * 128]
            wb_base[:, _WK + (ki * 2 + ko) * 128 : _WK + (ki * 2 + ko + 1) * 128] = \
                wkT[ki * 128 : (ki + 1) * 128, ko * 128 : (ko + 1) * 128]
        wb_base[:, _WV + ki * 256 : _WV + (ki + 1) * 256] = \
            wv.T[ki * 128 : (ki + 1) * 128, :]
        wb_base[:, _WO + ki * 256 : _WO + (ki + 1) * 256] = \
            wo.T[ki * 128 : (ki + 1) * 128, :]
    wb_base[:, _ONES : _ONES + 32] = 1.0

    # band mask [128, 96]: key row rho (padded coords), query col c:
    # valid iff c <= rho <= c + WHALF
    rho = np.arange(128)[:, None]
    c = np.arange(QT)[None, :]
    band = ((rho >= c) & (rho <= c + WHALF)).astype(np.float32)
    band_rep = np.tile(band, (1, 8))
    band0_c0 = np.tile(band * (rho >= PAD), (1, 8))  # core 0: no halo
    wm_base = np.zeros((128, _WBM_COLS), np.float32)
    wm_base[:, _BAND : _BAND + 8 * QT] = band_rep

    wf = np.zeros((128, 2), np.float32)
    for ko in range(2):
        wf[:, ko] = bq[ko * 128 : (ko + 1) * 128]

    qf = np.ascontiguousarray(query.transpose(2, 1, 0)).astype(bf)   # [E, N, L]
    kf = np.ascontiguousarray(key.transpose(2, 1, 0)).astype(bf)
    vf = np.ascontiguousarray(value.transpose(2, 1, 0)).astype(bf)

    in_maps = []
    for cidx in range(NCORES):
        l0 = cidx * T
        xq = qf[:, :, l0 : l0 + T].reshape(2, 128, N * T)
        xk = np.zeros((2, 128, N, TLP), bf)
        xv = np.zeros((2, 128, N, TLP), bf)
        kfc = kf.reshape(2, 128, N, L)
        vfc = vf.reshape(2, 128, N, L)
        xk[:, :, :, PAD : PAD + T] = kfc[:, :, :, l0 : l0 + T]
        xv[:, :, :, PAD : PAD + T] = vfc[:, :, :, l0 : l0 + T]
        if cidx > 0:
            xk[:, :, :, :PAD] = kfc[:, :, :, l0 - PAD : l0]
            xv[:, :, :, :PAD] = vfc[:, :, :, l0 - PAD : l0]
        wm = wm_base.copy()
        wm[:, _BAND0 : _BAND0 + 8 * QT] = band0_c0 if cidx == 0 else band_rep
        in_maps.append(
            {
                "xq": np.ascontiguousarray(xq),
                "xk": np.ascontiguousarray(xk.reshape(2, 128, N * TLP)),
                "xv": np.ascontiguousarray(xv.reshape(2, 128, N * TLP)),
                "wb": wb_base.astype(bf),
                "wm": wm.astype(bf),
                "wf": wf,
            }
        )
    add_vec = (out_proj_b + bv @ wo.T).astype(np.float32)
    return in_maps, add_vec


def _get_state():
    if "nc" not in _STATE:
        _STATE["nc"] = _build_program()
    return _STATE["nc"]


def kernel(query, key, value, in_proj_w, in_proj_b, out_proj_w, out_proj_b,
           collect_intermediates=0, _trace=False):
    from concourse.bass_utils import run_bass_kernel_spmd

    nc = _get_state()
    in_maps, add_vec = _host_prep(
        np.asarray(query), np.asarray(key), np.asarray(value),
        np.asarray(in_proj_w), np.asarray(in_proj_b),
        np.asarray(out_proj_w), np.asarray(out_proj_b),
    )
    res = run_bass_kernel_spmd(nc, in_maps, list(range(NCORES)), trace=_trace)
    out = np.empty((L, N, E), np.float32)
    for cidx in range(NCORES):
        dev = res.results[cidx]["out"]  # [NT, QT, N, E] bf16
        l0 = cidx * T
        for t in range(NT):
            q0 = QT * t
            qw = min(QT, T - q0)
            out[l0 + q0 : l0 + q0 + qw] = dev[t, :qw].astype(np.float32)
    out += add_vec
    if _trace:
        _STATE["last_exec_ns"] = res.exec_time_ns
        _STATE["last_res"] = res
    return out
